# revision 44
# baseline (speedup 1.0000x reference)
"""Trainium2 Bass kernel for nn_Encoder_base (5x ChebConv GNN + pool + MLP).

Distribution over 8 NeuronCores (fp16 data path, fp32 PSUM accumulation):
  - level-0 prop1: edge-sharded by destination; source rows PRE-GATHERED on
    the host (x is a kernel input), selection matrices host-built in fp16
  - level-0 prop2 + level-1 props: destination-sharded with full-width
    (all-batch) 2KB node rows -> few fat dma_gather indices instead of many
    thin ones; AllGather of z1/t1 between stages; a tiny dummy AllGather at
    kernel start absorbs the one-time collective-init barrier
  - einsums (channel mixes) node-sharded, 8 batch-group passes each
  - level 1 -> 2 handoff via AllToAll (node-shard -> batch-shard)
  - levels 2-3: batch-sharded dense-S matmuls in fp16
  - MLP: output-feature sharded; x-tiles stationary, fp16 weights as the
    N=512 moving operand; weights preloaded into SBUF over the SWDGE queue
    during the GNN phase; activations exchanged batch-major and re-tiled
    with transpose-mode dma_gather; BatchNorm stats via ones-matmul;
    final BN in fp32
"""
import numpy as np
import concourse.bass as bass
import concourse.bacc as bacc
import concourse.tile as tile
from concourse import mybir, bass_utils
from concourse.masks import make_identity

F32 = mybir.dt.float32
BF = mybir.dt.float16
I32 = mybir.dt.int32
I16 = mybir.dt.int16
NPBF = mybir.dt.np(BF)
AF = mybir.ActivationFunctionType
ALU = mybir.AluOpType
AX = mybir.AxisListType
RG = [list(range(8))]
NCORES = 8
N0, N1, N2, N3 = 16384, 4096, 1024, 128
EPS = 1e-5

_CACHE = {}


# ---------------------------------------------------------------- host prep
def _prep_prop(row, col, we, n_dest, n_shard):
    """Sorted-by-dest edges -> 128-dest windows, 128-edge chunks, padded so
    chunk counts per window match across shards (one SPMD program)."""
    window = 128
    order = np.argsort(row, kind="stable")
    row, col, we = row[order], col[order], we[order]
    per = n_dest // n_shard
    nwin = per // window
    counts = np.zeros((n_shard, nwin), np.int64)
    lists = {}
    for s in range(n_shard):
        lo = s * per
        for wi in range(nwin):
            wlo = lo + wi * window
            a = np.searchsorted(row, wlo, side="left")
            b = np.searchsorted(row, wlo + window, side="left")
            lists[(s, wi)] = (row[a:b] - wlo, col[a:b], we[a:b])
            counts[s, wi] = (b - a + 127) // 128
    ncw = np.maximum(counts.max(axis=0), 1)
    C = int(ncw.sum())
    src = np.zeros((n_shard, C, 128), np.int32)
    dst = np.full((n_shard, C, 128), 200.0, np.float32)
    wea = np.zeros((n_shard, C, 128), np.float32)
    for s in range(n_shard):
        base = 0
        for wi in range(nwin):
            dl, cl, wl = lists[(s, wi)]
            n = len(dl)
            k = int(ncw[wi])
            src[s, base:base + k].reshape(-1)[:n] = cl
            dst[s, base:base + k].reshape(-1)[:n] = dl
            wea[s, base:base + k].reshape(-1)[:n] = wl
            base += k
    return [int(x) for x in ncw], src, dst, wea


def _edge_we(e, n):
    row, col = np.asarray(e[0], np.int64), np.asarray(e[1], np.int64)
    deg = np.bincount(row, minlength=n).astype(np.float32)
    dis = np.where(deg > 0, 1.0 / np.sqrt(np.maximum(deg, 1.0)), 0.0).astype(np.float32)
    return row, col, -(dis[row] * dis[col]).astype(np.float32)


def _sub_edges(row, col, we, pool_idx):
    order = np.argsort(row, kind="stable")
    row, col, we = row[order], col[order], we[order]
    starts = np.searchsorted(row, pool_idx, side="left")
    ends = np.searchsorted(row, pool_idx, side="right")
    nr, ncl, nw = [], [], []
    for i in range(len(pool_idx)):
        s, e = starts[i], ends[i]
        if e > s:
            nr.append(np.full(e - s, i, np.int64))
            ncl.append(col[s:e])
            nw.append(we[s:e])
    return np.concatenate(nr), np.concatenate(ncl), np.concatenate(nw)


def _dense_s(row, col, we, n):
    s = np.zeros((n, n), np.float32)
    np.add.at(s, (row, col), we)
    return s


def _tile_w(w, pack):
    """[K, M] -> [K//(128*pack) * 128, pack*M]: pack K-blocks side by side."""
    k, m = w.shape
    nb = k // 128
    t = w.reshape(nb // pack, pack, 128, m).transpose(0, 2, 1, 3)
    return np.ascontiguousarray(t.reshape((nb // pack) * 128, pack * m))


def _sel_pack(dst, wea):
    """dst/wea [C, 128] -> bf16 selection blocks [128, C*128]."""
    C = dst.shape[0]
    sel = np.zeros((C, 128, 128), np.float32)
    c_idx, p_idx = np.meshgrid(np.arange(C), np.arange(128), indexing="ij")
    valid = dst < 128
    sel[c_idx[valid], p_idx[valid], dst[valid].astype(np.int64)] = wea[valid]
    return np.ascontiguousarray(
        sel.transpose(1, 0, 2).reshape(128, C * 128)).astype(NPBF)


def _rows_pack(table, src, width):
    """Pre-gathered rows: table [N, width], src [C, 128] -> [128, C*width]."""
    C = src.shape[0]
    g = table[src.reshape(-1)].reshape(C, 128, width)
    return np.ascontiguousarray(
        g.transpose(1, 0, 2).reshape(128, C * width)).astype(NPBF)


def _idx_pack(flat):
    return np.ascontiguousarray(
        np.tile(flat.astype(np.int16).reshape(-1, 16).T, (8, 1)))


def _host_prep(inputs):
    d = {k: np.asarray(v) for k, v in inputs.items()}
    x = d["x"].astype(np.float32)
    l0 = np.asarray(d["l0"], np.int64)
    l1 = np.asarray(d["l1"], np.int64)
    l2 = np.asarray(d["l2"], np.int64)

    X0 = np.ascontiguousarray(x.transpose(1, 0, 2).reshape(N0, 96))
    X0bf = X0.astype(NPBF)

    r0, c0, w0 = _edge_we(d["e0"], N0)
    ncw_p1, src_p1, dst_p1, we_p1 = _prep_prop(r0, c0, w0, N0, NCORES)
    r0s, c0s, w0s = _sub_edges(r0, c0, w0, l0)
    ncw_p2, src_p2, dst_p2, we_p2 = _prep_prop(r0s, c0s, w0s, N1, NCORES)

    r1, c1, w1 = _edge_we(d["e1"], N1)
    ncw_q1, src_q1, dst_q1, we_q1 = _prep_prop(r1, c1, w1, N1, NCORES)
    r1s, c1s, w1s = _sub_edges(r1, c1, w1, l1)
    ncw_q2, src_q2, dst_q2, we_q2 = _prep_prop(r1s, c1s, w1s, N2, NCORES)

    r2, c2, w2 = _edge_we(d["e2"], N2)
    S2 = _dense_s(r2, c2, w2, N2)
    S2T = _tile_w(np.ascontiguousarray(S2.T), 8).astype(NPBF)       # [128, 8192]
    S2l2T = _tile_w(np.ascontiguousarray(S2[l2].T), 8).astype(NPBF)  # [128, 1024]
    P_l2 = np.zeros((N2, 128), np.float32)
    P_l2[l2, np.arange(128)] = 1.0
    P_l2 = _tile_w(P_l2, 8).astype(NPBF)                             # [128, 1024]

    r3, c3, w3 = _edge_we(d["e3"], N3)
    S3T = np.ascontiguousarray(_dense_s(r3, c3, w3, N3).T).astype(NPBF)

    def wmod(W):
        return W[0] - W[2], W[1], 2.0 * W[2]

    Wm1 = wmod(d["Wc1"].astype(np.float32))
    Wm = [wmod(d[f"Wc{i}"].astype(np.float32)) for i in (2, 3, 4, 5)]
    eye4 = np.eye(4, dtype=np.float32)

    per_core = []
    for k in range(NCORES):
        m = {}
        m["epsv"] = np.full((128, 1), EPS, np.float32)
        # ---- p1: host-gathered x rows + host sel blocks
        m["p1_xg"] = _rows_pack(X0bf, src_p1[k], 96)
        m["p1_sel"] = _sel_pack(dst_p1[k], we_p1[k])
        # ---- p2: gather idx (into tx1_all) + sel
        m["p2_idx"] = _idx_pack(src_p2[k].reshape(-1))
        m["p2_sel"] = _sel_pack(dst_p2[k], we_p2[k])
        # ---- q1 / q2
        m["q1_idx"] = _idx_pack(src_q1[k].reshape(-1))
        m["q1_sel"] = _sel_pack(dst_q1[k], we_q1[k])
        m["q2_idx"] = _idx_pack(src_q2[k].reshape(-1))
        m["q2_sel"] = _sel_pack(dst_q2[k], we_q2[k])
        # ---- einsum l0 (node shard 512k..512k+512)
        l0s = l0[512 * k:512 * (k + 1)]
        m["g0T"] = np.ascontiguousarray(X0[l0s].T).astype(NPBF)  # [96, 512]
        m["l0_idx"] = _idx_pack(l0s)
        for g in range(8):
            for t in range(3):
                bw = np.zeros((96, 128), np.float32)
                for j in range(4):
                    b = 4 * g + j
                    bw[3 * b:3 * b + 3, 32 * j:32 * j + 32] = Wm1[t]
                m[f"bw0_{g}_{t}"] = bw.astype(NPBF)
        # ---- einsum l1 (node shard 128k..128k+128)
        m["l1_idx"] = _idx_pack(l1[128 * k:128 * (k + 1)])
        for lev in range(4):
            for t in range(3):
                m[f"bigw{lev + 1}_{t}"] = np.kron(eye4, Wm[lev][t]).astype(NPBF)
        for lev, nm in ((1, "b1"), (2, "b2"), (3, "b3"), (4, "b4"), (5, "b5")):
            m[f"bias{lev}"] = np.tile(d[nm].astype(np.float32), 4).reshape(128, 1)
        # ---- level 2/3 dense
        m["S2T"] = S2T
        m["S2l2T"] = S2l2T
        m["P_l2"] = P_l2
        m["S3T"] = S3T
        # ---- MLP (feature shard 512k..512k+512)
        m["ones32"] = np.ones((32, 1), np.float32)
        m["one1x32"] = np.ones((1, 32), np.float32)
        for li in (6, 7, 8):
            W = d[f"W{li}"].astype(np.float32)[:, 512 * k:512 * k + 512]
            m[f"w{li}"] = _tile_w(W, 8).astype(NPBF)  # [512, 4096]
            m[f"gb{li}"] = d[f"g{li}"].astype(np.float32)[
                512 * k:512 * k + 512].reshape(1, 512).copy()
            m[f"bb{li}"] = d[f"be{li}"].astype(np.float32)[
                512 * k:512 * k + 512].reshape(1, 512).copy()
        m["w9"] = _tile_w(
            d["W9"].astype(np.float32)[512 * k:512 * k + 512], 4).astype(NPBF)
        # transpose-gather index tables for MLP activations
        m["x6g_idx"] = _idx_pack(np.array(
            [b * 4 + q for q in range(4) for b in range(32)], np.int64))
        m["h_idx"] = _idx_pack(np.arange(256, dtype=np.int64))
        per_core.append(m)

    meta = {"p1": ncw_p1, "p2": ncw_p2, "q1": ncw_q1, "q2": ncw_q2}
    return per_core, meta


# ---------------------------------------------------------------- device program
def _build_nc(meta, shapes):
    nc = bacc.Bacc("TRN2", target_bir_lowering=False, debug=False, num_devices=NCORES)
    dtmap = {np.dtype(np.int32): I32, np.dtype(np.int16): I16,
             np.dtype(NPBF): BF, np.dtype(np.float32): F32}
    ein = {}
    for name, arr in shapes.items():
        ein[name] = nc.dram_tensor(name, list(arr.shape), dtmap[arr.dtype],
                                   kind="ExternalInput")
    out_mu = nc.dram_tensor("mu", [128, 32], F32, kind="ExternalOutput")

    tx1_loc = nc.dram_tensor("tx1_loc", [N0 // 8, 128], BF)
    tx1_all = nc.dram_tensor("tx1_all", [N0, 128], BF, addr_space="Shared")
    z1_loc = nc.dram_tensor("z1_loc", [512, 1024], BF)
    z1_all = nc.dram_tensor("z1_all", [N1, 1024], BF, addr_space="Shared")
    t1_loc = nc.dram_tensor("t1_loc", [512, 1024], BF)
    t1_all = nc.dram_tensor("t1_all", [N1, 1024], BF, addr_space="Shared")
    z2_a2a_in = nc.dram_tensor("z2_a2a_in", [1024, 128], BF)
    z2_a2a_out = nc.dram_tensor("z2_a2a_out", [1024, 128], BF)
    x6_loc = nc.dram_tensor("x6_loc", [16, 1024], BF)
    x6_all = nc.dram_tensor("x6_all", [128, 1024], BF, addr_space="Shared")
    h6_loc = nc.dram_tensor("h6_loc", [32, 512], BF)
    h6_all = nc.dram_tensor("h6_all", [256, 512], BF, addr_space="Shared")
    h7_loc = nc.dram_tensor("h7_loc", [32, 512], BF)
    h7_all = nc.dram_tensor("h7_all", [256, 512], BF, addr_space="Shared")
    dmy_loc = nc.dram_tensor("dmy_loc", [16, 16], BF)
    dmy_all = nc.dram_tensor("dmy_all", [128, 16], BF, addr_space="Shared")
    mu_loc = nc.dram_tensor("mu_loc", [128, 32], F32)
    mu_all = nc.dram_tensor("mu_all", [8 * 128, 32], F32, addr_space="Shared")

    C1 = sum(meta["p1"])
    C2 = sum(meta["p2"])
    C3 = sum(meta["q1"])
    C4 = sum(meta["q2"])

    with tile.TileContext(nc) as tc:
        with (
            tc.tile_pool(name="const", bufs=1) as cpool,
            tc.tile_pool(name="grp", bufs=2) as gpool,
            tc.tile_pool(name="zb", bufs=3) as zpool,
            tc.tile_pool(name="work", bufs=3) as wpool,
            tc.tile_pool(name="wbig", bufs=2) as wbpool,
            tc.tile_pool(name="ps_s", bufs=2, space="PSUM") as pps,
        ):
            identf = cpool.tile([128, 128], F32, tag="identf", name="identf")
            make_identity(nc, identf[:])
            identb = cpool.tile([128, 128], BF, tag="identb", name="identb")
            nc.vector.tensor_copy(identb[:], identf[:])
            eps_t = cpool.tile([128, 1], F32, tag="epsv", name="epsv")
            nc.sync.dma_start(out=eps_t[:], in_=ein["epsv"][:, :])

            # big weight preloads ride the SWDGE queue: transfers overlap the
            # GNN phase without blocking the HWDGE rings that feed it
            def preload_w(nm):
                halves = []
                for hh in range(2):
                    t = wbpool.tile([128, 8192], BF, tag="wbig", name="wbig")
                    nc.gpsimd.dma_start(
                        out=t[:].rearrange("p (i f) -> p i f", f=4096),
                        in_=ein[nm].ap().rearrange("(i p) f -> p i f", p=128)
                        [:, 2 * hh:2 * hh + 2, :])
                    halves.append(t)
                return halves

            w6sb = preload_w("w6")
            s2t_sb = cpool.tile([128, 8192], BF, tag="S2T", name="S2T")
            nc.gpsimd.dma_start(out=s2t_sb[:], in_=ein["S2T"][:, :])
            # fire a tiny collective immediately: absorbs the one-time
            # collective-comm init barrier (~60us) behind the level-0 compute
            nc.gpsimd.collective_compute(
                "AllGather", ALU.bypass, replica_groups=RG,
                ins=[dmy_loc.ap().opt()], outs=[dmy_all.ap().opt()])

            def load_const(name, dt=BF):
                t = cpool.tile(list(shapes[name].shape), dt, tag=name)
                nc.sync.dma_start(out=t[:], in_=ein[name][:, :])
                return t

            def load_idx(name, ncols):
                t = cpool.tile([128, ncols], I16, tag=name, name=name)
                nc.sync.dma_start(out=t[:], in_=ein[name][:, :])
                return t

            # group loader for host-packed per-chunk arrays ([128, C*w] in DRAM)
            def mk_loader(ein_name, w, nchunks, grp, tag, eng):
                tiles = {}

                def get(cc):
                    g0 = (cc // grp) * grp
                    if g0 not in tiles:
                        gc = min(grp, nchunks - g0)
                        t = gpool.tile([128, grp * w], BF, tag=tag, name=tag)
                        eng.dma_start(out=t[:, :gc * w],
                                      in_=ein[ein_name][:, g0 * w:(g0 + gc) * w])
                        tiles[g0] = t
                    return tiles[g0], (cc % grp) * w
                return get

            # gather groups: idx_sb [128, nchunks*8] (128 idx per chunk)
            def mk_gather(idx_sb, src_dram, w, nchunks, grp, tag, bufs=3):
                tiles = {}

                def get(cc):
                    g0 = (cc // grp) * grp
                    if g0 not in tiles:
                        gc = min(grp, nchunks - g0)
                        t = zpool.tile([128, grp * w], BF, tag=tag, name=tag, bufs=bufs)
                        nc.gpsimd.dma_gather(
                            out_ap=t[:, :gc * w].rearrange("p (c e) -> p c e", e=w),
                            in_ap=src_dram[:, :],
                            idxs_ap=idx_sb[:, g0 * 8:(g0 + gc) * 8],
                            num_idxs=gc * 128, num_idxs_reg=gc * 128, elem_size=w,
                            single_packet=False)
                        tiles[g0] = t
                    return tiles[g0], (cc % grp) * w

                def trigger():
                    pass
                return get, trigger

            def transp(src_ap, dst_ap):
                p, f = src_ap.shape
                b0 = src_ap.base_partition()
                ps = pps.tile([128, 128], BF, tag="tps", name="tps")
                nc.tensor.transpose(out=ps[:f, :p], in_=src_ap,
                                    identity=identb[b0:b0 + p, b0:b0 + p])
                nc.scalar.activation(out=dst_ap, in_=ps[:f, :p], func=AF.Copy)

            # ================= LEVEL 0: prop1 (host-gathered sources) ========
            p2i = load_idx("p2_idx", C2 * 8)
            gz, gz_fire = mk_gather(p2i, tx1_all, 128, C2, 16, "p2zb", bufs=2)
            with nc.named_scope("l0_prop1"):
                xg = mk_loader("p1_xg", 96, C1, 16, "p1xg", nc.sync)
                sl = mk_loader("p1_sel", 128, C1, 16, "p1sel", nc.scalar)
                base = 0
                for wi, nch in enumerate(meta["p1"]):
                    ps = pps.tile([128, 512], F32, tag="pp1", name="pp1")
                    for c in range(nch):
                        cc = base + c
                        xt, xo = xg(cc)
                        st, so = sl(cc)
                        nc.tensor.matmul(out=ps[:, :96],
                                         lhsT=st[:, so:so + 128],
                                         rhs=xt[:, xo:xo + 96],
                                         start=(c == 0), stop=(c == nch - 1))
                    ev = wpool.tile([128, 96], BF, tag="p1ev", name="p1ev", bufs=4)
                    nc.vector.tensor_copy(ev[:], ps[:, :96])
                    nc.sync.dma_start(out=tx1_loc[wi * 128:(wi + 1) * 128, :96], in_=ev[:])
                    base += nch
            with nc.named_scope("ag1"):
                nc.gpsimd.collective_compute(
                    "AllGather", ALU.bypass, replica_groups=RG,
                    ins=[tx1_loc.ap().opt()], outs=[tx1_all.ap().opt()])
                gz_fire()

            # ================= LEVEL 0: prop2 (dest = own l0 shard) ==========
            p2T_sb = cpool.tile([96, 512], BF, tag="p2T_sb", name="p2T_sb")
            with nc.named_scope("l0_prop2"):
                sl2 = mk_loader("p2_sel", 128, C2, 16, "p2sel", nc.scalar)
                base = 0
                for wi, nch in enumerate(meta["p2"]):
                    ps = pps.tile([128, 512], F32, tag="pp1", name="pp1")
                    for c in range(nch):
                        cc = base + c
                        zt, zo = gz(cc)
                        st, so = sl2(cc)
                        nc.tensor.matmul(out=ps[:96, :128],
                                         lhsT=zt[:, zo:zo + 96],
                                         rhs=st[:, so:so + 128],
                                         start=(c == 0), stop=(c == nch - 1))
                    nc.scalar.activation(out=p2T_sb[:, wi * 128:(wi + 1) * 128],
                                         in_=ps[:96, :128], func=AF.Copy)
                    base += nch

            # ================= LEVEL 0: einsum -> z1 =========================
            with nc.named_scope("l0_einsum"):
                g0T = load_const("g0T")                      # [96, 512]
                l0i = load_idx("l0_idx", 32)
                gz1, gz1_fire = mk_gather(l0i, tx1_all, 128, 4, 4, "g1zb", bufs=1)
                gz1_fire()
                g1T = cpool.tile([96, 512], BF, tag="g1T", name="g1T")
                for c in range(4):
                    zt, zo = gz1(c)
                    transp(zt[:, zo:zo + 96], g1T[:, c * 128:(c + 1) * 128])
                bias1 = load_const("bias1", F32)
                for g in range(8):
                    bw = [load_const(f"bw0_{g}_{t}") for t in range(3)]
                    ps = pps.tile([128, 512], F32, tag="pp1", name="pp1")
                    for t, tap in enumerate((g0T, g1T, p2T_sb)):
                        nc.tensor.matmul(out=ps[:, :512], lhsT=bw[t][:, :],
                                         rhs=tap[:, :], start=(t == 0), stop=(t == 2))
                    z1Tg = wpool.tile([128, 512], BF, tag="z1Tg", name="z1Tg", bufs=2)
                    nc.scalar.activation(out=z1Tg[:], in_=ps[:, :512],
                                         func=AF.Identity, bias=bias1[:, 0:1])
                    z1g = wpool.tile([128, 512], BF, tag="z1g", name="z1g", bufs=2)
                    for c in range(4):
                        transp(z1Tg[:, c * 128:(c + 1) * 128],
                               z1g[:, c * 128:(c + 1) * 128])
                    nc.sync.dma_start(
                        out=z1_loc[:, g * 128:(g + 1) * 128].rearrange(
                            "(c p) f -> p c f", p=128),
                        in_=z1g[:].rearrange("p (c f) -> p c f", f=128))
            q1i = load_idx("q1_idx", C3 * 8)
            gq1, gq1_fire = mk_gather(q1i, z1_all, 1024, C3, 4, "q1zb", bufs=2)
            l1i = load_idx("l1_idx", 8)
            gzl1, gzl1_fire = mk_gather(l1i, z1_all, 1024, 1, 1, "el1a", bufs=1)
            with nc.named_scope("ag_z1"):
                nc.gpsimd.collective_compute(
                    "AllGather", ALU.bypass, replica_groups=RG,
                    ins=[z1_loc.ap().opt()], outs=[z1_all.ap().opt()])
                gq1_fire()

            # ================= LEVEL 1: prop1 (dest-sharded, fat rows) =======
            with nc.named_scope("l1_prop1"):
                slq1 = mk_loader("q1_sel", 128, C3, 8, "q1sel", nc.scalar)
                base = 0
                for wi, nch in enumerate(meta["q1"]):
                    psh = [pps.tile([128, 512], F32, tag="pp1", name="pp1")
                           for _ in range(2)]
                    for c in range(nch):
                        cc = base + c
                        zt, zo = gq1(cc)
                        st, so = slq1(cc)
                        for h in range(2):
                            nc.tensor.matmul(
                                out=psh[h][:, :512],
                                lhsT=st[:, so:so + 128],
                                rhs=zt[:, zo + h * 512:zo + (h + 1) * 512],
                                start=(c == 0), stop=(c == nch - 1))
                    ev = wpool.tile([128, 1024], BF, tag="q1ev", name="q1ev", bufs=2)
                    for h in range(2):
                        nc.scalar.activation(out=ev[:, h * 512:(h + 1) * 512],
                                             in_=psh[h][:, :512], func=AF.Copy)
                    nc.sync.dma_start(out=t1_loc[wi * 128:(wi + 1) * 128, :], in_=ev[:])
                    base += nch
                # z1 einsum taps don't depend on t1: transpose them early
                z1l1T = cpool.tile([128, 1024], BF, tag="z1l1T", name="z1l1T")
                zt_l1, _ = gzl1(0)
                for g in range(8):
                    transp(zt_l1[:, g * 128:(g + 1) * 128],
                           z1l1T[:, g * 128:(g + 1) * 128])
            gtl1, gtl1_fire = mk_gather(l1i, t1_all, 1024, 1, 1, "el1b", bufs=1)
            q2i = load_idx("q2_idx", C4 * 8)
            gq2, gq2_fire = mk_gather(q2i, t1_all, 1024, C4, 4, "q1zb", bufs=2)
            with nc.named_scope("ag_t1"):
                nc.gpsimd.collective_compute(
                    "AllGather", ALU.bypass, replica_groups=RG,
                    ins=[t1_loc.ap().opt()], outs=[t1_all.ap().opt()])
                gtl1_fire()

            # ================= LEVEL 1: prop2 (dest = own l1 shard) ==========
            p2q = cpool.tile([128, 1024], BF, tag="p2q", name="p2q")
            with nc.named_scope("l1_prop2"):
                tt_l1, _ = gtl1(0)
                slq2 = mk_loader("q2_sel", 128, C4, 8, "q2sel", nc.scalar)
                psh = [pps.tile([128, 512], F32, tag="pp1", name="pp1")
                       for _ in range(2)]
                for c in range(C4):
                    zt, zo = gq2(c)
                    st, so = slq2(c)
                    for h in range(2):
                        nc.tensor.matmul(
                            out=psh[h][:, :512],
                            lhsT=st[:, so:so + 128],
                            rhs=zt[:, zo + h * 512:zo + (h + 1) * 512],
                            start=(c == 0), stop=(c == C4 - 1))
                for h in range(2):
                    nc.scalar.activation(out=p2q[:, h * 512:(h + 1) * 512],
                                         in_=psh[h][:, :512], func=AF.Copy)

            # ================= LEVEL 1: einsum -> z2 =========================
            with nc.named_scope("l1_einsum"):
                t1l1T = cpool.tile([128, 1024], BF, tag="t1l1T", name="t1l1T")
                p2qT = cpool.tile([128, 1024], BF, tag="p2qT", name="p2qT")
                for g in range(8):
                    transp(tt_l1[:, g * 128:(g + 1) * 128], t1l1T[:, g * 128:(g + 1) * 128])
                    transp(p2q[:, g * 128:(g + 1) * 128], p2qT[:, g * 128:(g + 1) * 128])
                bw1 = [load_const(f"bigw1_{t}") for t in range(3)]
                bias2 = load_const("bias2", F32)
                for g in range(8):
                    ps = pps.tile([128, 512], F32, tag="pp1", name="pp1")
                    for t, tap in enumerate((z1l1T, t1l1T, p2qT)):
                        nc.tensor.matmul(out=ps[:, :128], lhsT=bw1[t][:, :],
                                         rhs=tap[:, g * 128:(g + 1) * 128],
                                         start=(t == 0), stop=(t == 2))
                    z2Tg = wpool.tile([128, 128], BF, tag="z2Tg", name="z2Tg")
                    nc.scalar.activation(out=z2Tg[:], in_=ps[:, :128],
                                         func=AF.Tanh, bias=bias2[:, 0:1])
                    z2ng = wpool.tile([128, 128], BF, tag="z2ng", name="z2ng", bufs=4)
                    transp(z2Tg[:], z2ng[:])
                    nc.sync.dma_start(out=z2_a2a_in[g * 128:(g + 1) * 128, :],
                                      in_=z2ng[:])
            with nc.named_scope("a2a_z2"):
                nc.gpsimd.collective_compute(
                    "AllToAll", ALU.bypass, replica_groups=RG,
                    ins=[z2_a2a_in.ap().opt()], outs=[z2_a2a_out.ap().opt()])

            # ================= LEVEL 2 (dense, batch-sharded) ================
            with nc.named_scope("l2"):
                # z2n: [128 nodes, 128 own-feats] tiles, k-tile kc
                z2n = cpool.tile([128, 1024], BF, tag="z2n", name="z2n")
                nc.sync.dma_start(
                    out=z2n[:].rearrange("p (c f) -> p c f", f=128),
                    in_=z2_a2a_out.ap().rearrange("(c p) f -> p c f", p=128))
                t1_l2 = cpool.tile([128, 1024], BF, tag="t1_l2", name="t1_l2")
                s2t = s2t_sb
                for dc in range(8):
                    ps = pps.tile([128, 512], F32, tag="pp1", name="pp1")
                    for kc in range(8):
                        nc.tensor.matmul(
                            out=ps[:, :128],
                            lhsT=s2t[:, kc * 1024 + dc * 128: kc * 1024 + dc * 128 + 128],
                            rhs=z2n[:, kc * 128:(kc + 1) * 128],
                            start=(kc == 0), stop=(kc == 7))
                    nc.scalar.activation(out=t1_l2[:, dc * 128:(dc + 1) * 128],
                                         in_=ps[:, :128], func=AF.Copy)
                s2l2 = load_const("S2l2T")
                ps = pps.tile([128, 512], F32, tag="pp1", name="pp1")
                for kc in range(8):
                    nc.tensor.matmul(out=ps[:, :128], lhsT=s2l2[:, kc * 128:(kc + 1) * 128],
                                     rhs=t1_l2[:, kc * 128:(kc + 1) * 128],
                                     start=(kc == 0), stop=(kc == 7))
                p2n_l2 = wpool.tile([128, 128], BF, tag="p2n_l2", name="p2n_l2")
                nc.scalar.activation(out=p2n_l2[:], in_=ps[:, :128], func=AF.Copy)
                pl2 = load_const("P_l2")
                z2l2T = wpool.tile([128, 128], BF, tag="z2l2T", name="z2l2T")
                psg = pps.tile([128, 512], F32, tag="pp1", name="pp1")
                for kc in range(8):
                    nc.tensor.matmul(out=psg[:, :128], lhsT=z2n[:, kc * 128:(kc + 1) * 128],
                                     rhs=pl2[:, kc * 128:(kc + 1) * 128],
                                     start=(kc == 0), stop=(kc == 7))
                nc.scalar.activation(out=z2l2T[:], in_=psg[:, :128], func=AF.Copy)
                t1l2T = wpool.tile([128, 128], BF, tag="t1l2T", name="t1l2T")
                psg2 = pps.tile([128, 512], F32, tag="pp1", name="pp1")
                for kc in range(8):
                    nc.tensor.matmul(out=psg2[:, :128], lhsT=t1_l2[:, kc * 128:(kc + 1) * 128],
                                     rhs=pl2[:, kc * 128:(kc + 1) * 128],
                                     start=(kc == 0), stop=(kc == 7))
                nc.scalar.activation(out=t1l2T[:], in_=psg2[:, :128], func=AF.Copy)
                p2l2T = wpool.tile([128, 128], BF, tag="p2l2T", name="p2l2T")
                transp(p2n_l2[:], p2l2T[:])
                bw2 = [load_const(f"bigw2_{t}") for t in range(3)]
                bias3 = load_const("bias3", F32)
                ps3 = pps.tile([128, 512], F32, tag="pp1", name="pp1")
                for t, tap in enumerate((z2l2T, t1l2T, p2l2T)):
                    nc.tensor.matmul(out=ps3[:, :128], lhsT=bw2[t][:, :], rhs=tap[:],
                                     start=(t == 0), stop=(t == 2))
                z3T = wpool.tile([128, 128], BF, tag="z3T", name="z3T")
                nc.scalar.activation(out=z3T[:], in_=ps3[:, :128],
                                     func=AF.Tanh, bias=bias3[:, 0:1])
                z3n = wpool.tile([128, 128], BF, tag="z3n", name="z3n")
                transp(z3T[:], z3n[:])

            # ================= LEVEL 3 =================
            with nc.named_scope("l3"):
                s3t = load_const("S3T")
                bias4 = load_const("bias4", F32)
                bias5 = load_const("bias5", F32)

                def conv_l3(zn, zT, bw_pref, bias_t, func, keep):
                    t1T = wpool.tile([128, 128], BF, tag=keep + "t1T", name=keep + "t1T")
                    ps = pps.tile([128, 512], F32, tag="pp1", name="pp1")
                    nc.tensor.matmul(out=ps[:, :128], lhsT=zn, rhs=s3t[:], start=True, stop=True)
                    nc.scalar.activation(out=t1T[:], in_=ps[:, :128], func=AF.Copy)
                    t1n_ = wpool.tile([128, 128], BF, tag=keep + "t1n", name=keep + "t1n")
                    transp(t1T[:], t1n_[:])
                    p2T_ = wpool.tile([128, 128], BF, tag=keep + "p2T", name=keep + "p2T")
                    ps2 = pps.tile([128, 512], F32, tag="pp1", name="pp1")
                    nc.tensor.matmul(out=ps2[:, :128], lhsT=t1n_[:], rhs=s3t[:], start=True, stop=True)
                    nc.scalar.activation(out=p2T_[:], in_=ps2[:, :128], func=AF.Copy)
                    bw = [load_const(f"{bw_pref}_{t}") for t in range(3)]
                    outT = wpool.tile([128, 128], BF, tag=keep + "oT", name=keep + "oT")
                    ps4 = pps.tile([128, 512], F32, tag="pp1", name="pp1")
                    for t, tap in enumerate((zT, t1T[:], p2T_[:])):
                        nc.tensor.matmul(out=ps4[:, :128], lhsT=bw[t][:, :], rhs=tap,
                                         start=(t == 0), stop=(t == 2))
                    f2 = AF.Identity if func == AF.Copy else func
                    nc.scalar.activation(out=outT[:], in_=ps4[:, :128], func=f2,
                                         bias=bias_t[:, 0:1])
                    outn = wpool.tile([128, 128], BF, tag=keep + "on", name=keep + "on")
                    transp(outT[:], outn[:])
                    return outn, outT

                z4n, z4T = conv_l3(z3n[:], z3T[:], "bigw3", bias4, AF.Tanh, "c4")
                o5n, o5T = conv_l3(z4n[:], z4T[:], "bigw4", bias5, AF.Copy, "c5")

            # ================= MLP input assembly (batch-major rows) =========
            with nc.named_scope("mlp_in"):
                # x6_loc rows 4*jb+q (jb=own batch, q=node quarter), 1024 feats
                for jb in range(4):
                    nc.sync.dma_start(
                        out=x6_loc.ap()[4 * jb:4 * jb + 4, :].rearrange(
                            "q (nn c) -> (q nn) c", c=32),
                        in_=o5n[:, 32 * jb:32 * jb + 32])
                nc.gpsimd.collective_compute(
                    "AllGather", ALU.bypass, replica_groups=RG,
                    ins=[x6_loc.ap().opt()], outs=[x6_all.ap().opt()])

            # ================= MLP =================
            ones32 = load_const("ones32", F32)
            one1x32 = load_const("one1x32", F32)

            def gather_xT(idx_t, table, elem, n_idx, tag):
                t = wpool.tile([128, 1024], BF, tag="xg", name=tag, bufs=2)
                nc.gpsimd.dma_gather(
                    out_ap=t[:].rearrange("p (c i) -> p c i", i=n_idx),
                    in_ap=table[:, :], idxs_ap=idx_t[:, :],
                    num_idxs=n_idx, num_idxs_reg=n_idx, elem_size=elem,
                    transpose=True, single_packet=False)
                return t

            def mlp_layer(nm, xg_of, wsb):
                gb = load_const("gb" + nm[1], F32)
                bb = load_const("bb" + nm[1], F32)
                acc = pps.tile([32, 512], F32, tag="macc", name="macc", bufs=1)
                for kc in range(32):
                    wt = wsb[kc // 16]
                    nc.tensor.matmul(out=acc[:, :512], lhsT=xg_of(kc),
                                     rhs=wt[:, (kc % 16) * 512:(kc % 16 + 1) * 512],
                                     start=(kc == 0), stop=(kc == 31))
                h = wpool.tile([32, 512], F32, tag="mh", name="mh", bufs=1)
                nc.vector.tensor_copy(h[:], acc[:, :512])
                hsq = wpool.tile([32, 512], F32, tag="mhsq", name="mhsq", bufs=1)
                nc.vector.tensor_mul(hsq[:], h[:], h[:])
                st1 = pps.tile([1, 512], F32, tag="mst", name="mst", bufs=2)
                nc.tensor.matmul(out=st1[:, :512], lhsT=ones32[:, :], rhs=h[:],
                                 start=True, stop=True)
                st2 = pps.tile([1, 512], F32, tag="mst", name="mst", bufs=2)
                nc.tensor.matmul(out=st2[:, :512], lhsT=ones32[:, :], rhs=hsq[:],
                                 start=True, stop=True)
                # stats all on partition 0: [mu | var | a | c]
                s = wpool.tile([1, 2048], F32, tag="mstat", name="mstat", bufs=1)
                mu_, va_, aa_, cc_ = (s[0:1, 512 * i:512 * (i + 1)] for i in range(4))
                nc.vector.tensor_scalar_mul(mu_, st1[:1, :512], 1.0 / 32.0)
                nc.vector.tensor_scalar_mul(va_, st2[:1, :512], 1.0 / 32.0)
                nc.vector.tensor_mul(aa_, mu_, mu_)
                nc.vector.tensor_tensor(va_, va_, aa_, op=ALU.subtract)
                nc.scalar.activation(out=aa_, in_=va_, func=AF.Sqrt,
                                     bias=eps_t[0:1, 0:1])
                nc.vector.reciprocal(aa_, aa_)
                nc.vector.tensor_mul(aa_, aa_, gb[:])
                nc.vector.tensor_mul(cc_, mu_, aa_)
                nc.vector.tensor_tensor(cc_, bb[:], cc_, op=ALU.subtract)
                # broadcast a/c to 32 batch partitions via K=1 matmuls
                pb = pps.tile([32, 512], F32, tag="macc", name="macc", bufs=1)
                nc.tensor.matmul(out=pb[:, :512], lhsT=one1x32[:, :], rhs=aa_,
                                 start=True, stop=True)
                ab = wpool.tile([32, 512], F32, tag="mab", name="mab", bufs=1)
                nc.vector.tensor_copy(ab[:], pb[:, :512])
                pb2 = pps.tile([32, 512], F32, tag="macc", name="macc", bufs=1)
                nc.tensor.matmul(out=pb2[:, :512], lhsT=one1x32[:, :], rhs=cc_,
                                 start=True, stop=True)
                cb = wpool.tile([32, 512], F32, tag="mcb", name="mcb", bufs=1)
                nc.vector.tensor_copy(cb[:], pb2[:, :512])
                ha = wpool.tile([32, 512], F32, tag="mha", name="mha", bufs=1)
                nc.vector.tensor_mul(ha[:], h[:], ab[:])
                nc.vector.tensor_add(ha[:], ha[:], cb[:])
                h16 = wpool.tile([32, 512], BF, tag="mh16", name="mh16", bufs=2)
                nc.vector.tensor_scalar(out=h16[:], in0=ha[:], scalar1=0.0,
                                        scalar2=None, op0=ALU.max)
                return h16

            with nc.named_scope("mlp6"):
                x6gi = load_idx("x6g_idx", 8)
                x6g = gather_xT(x6gi, x6_all, 1024, 128, "x6g")
                w7sb = preload_w("w7")
                h6 = mlp_layer("w6", lambda kc: x6g[:, (kc % 8) * 128 + (kc // 8) * 32:
                                                    (kc % 8) * 128 + (kc // 8) * 32 + 32],
                               w6sb)
                nc.sync.dma_start(out=h6_loc[:, :], in_=h6[:])
                nc.gpsimd.collective_compute(
                    "AllGather", ALU.bypass, replica_groups=RG,
                    ins=[h6_loc.ap().opt()], outs=[h6_all.ap().opt()])
            with nc.named_scope("mlp7"):
                hgi = load_idx("h_idx", 16)
                x7g = gather_xT(hgi, h6_all, 512, 256, "x7g")
                w8sb = preload_w("w8")
                h7 = mlp_layer("w7", lambda kc: x7g[:, (kc % 4) * 256 + (kc // 4) * 32:
                                                    (kc % 4) * 256 + (kc // 4) * 32 + 32],
                               w7sb)
                nc.sync.dma_start(out=h7_loc[:, :], in_=h7[:])
                nc.gpsimd.collective_compute(
                    "AllGather", ALU.bypass, replica_groups=RG,
                    ins=[h7_loc.ap().opt()], outs=[h7_all.ap().opt()])
            with nc.named_scope("mlp8"):
                x8g = gather_xT(hgi, h7_all, 512, 256, "x8g")
                h8 = mlp_layer("w8", lambda kc: x8g[:, (kc % 4) * 256 + (kc // 4) * 32:
                                                    (kc % 4) * 256 + (kc // 4) * 32 + 32],
                               w8sb)

            with nc.named_scope("mlp9"):
                w9t = load_const("w9")
                x9 = wpool.tile([128, 128], BF, tag="x9", name="x9")
                for kc in range(4):
                    transp(h8[:, kc * 128:(kc + 1) * 128], x9[:, 32 * kc:32 * kc + 32])
                ps9 = pps.tile([128, 128], F32, tag="mac9", name="mac9", bufs=1)
                for kc in range(4):
                    nc.tensor.matmul(out=ps9[:, :32], lhsT=w9t[:, kc * 128:(kc + 1) * 128],
                                     rhs=x9[:, 32 * kc:32 * kc + 32],
                                     start=(kc == 0), stop=(kc == 3))
                mu_sb = wpool.tile([128, 32], F32, tag="mu_sb", name="mu_sb")
                nc.vector.tensor_copy(mu_sb[:], ps9[:, :32])
                nc.sync.dma_start(out=mu_loc[:, :], in_=mu_sb[:])
                nc.gpsimd.collective_compute(
                    "AllGather", ALU.bypass, replica_groups=RG,
                    ins=[mu_loc.ap().opt()], outs=[mu_all.ap().opt()])
                mall = wpool.tile([128, 256], F32, tag="f_mall", name="f_mall")
                nc.sync.dma_start(
                    out=mall[:].rearrange("p (k b) -> p k b", b=32),
                    in_=mu_all.ap().rearrange("(k p) b -> p k b", p=128))
                tot = wpool.tile([128, 32], F32, tag="f_tot", name="f_tot")
                nc.vector.tensor_copy(tot[:], mall[:, 0:32])
                for k in range(1, 8):
                    nc.vector.tensor_add(tot[:], tot[:], mall[:, 32 * k:32 * k + 32])
                s1 = wpool.tile([128, 1], F32, tag="f_s1", name="f_s1")
                nc.vector.tensor_reduce(out=s1[:], in_=tot[:], axis=AX.X, op=ALU.add)
                mu_ = wpool.tile([128, 1], F32, tag="f_mu", name="f_mu")
                nc.vector.tensor_scalar_mul(mu_[:], s1[:], 1.0 / 32.0)
                sq = wpool.tile([128, 32], F32, tag="f_sq", name="f_sq")
                nc.vector.tensor_mul(sq[:], tot[:], tot[:])
                s2_ = wpool.tile([128, 1], F32, tag="f_s2", name="f_s2")
                nc.vector.tensor_reduce(out=s2_[:], in_=sq[:], axis=AX.X, op=ALU.add)
                var = wpool.tile([128, 1], F32, tag="f_var", name="f_var")
                nc.vector.scalar_tensor_tensor(out=var[:], in0=mu_[:], scalar=-1.0,
                                               in1=mu_[:], op0=ALU.mult, op1=ALU.mult)
                nc.vector.scalar_tensor_tensor(out=var[:], in0=s2_[:], scalar=1.0 / 32.0,
                                               in1=var[:], op0=ALU.mult, op1=ALU.add)
                sdf = wpool.tile([128, 1], F32, tag="f_sd", name="f_sd")
                nc.scalar.activation(out=sdf[:], in_=var[:], func=AF.Sqrt, bias=eps_t[:, 0:1])
                rs = wpool.tile([128, 1], F32, tag="f_rs", name="f_rs")
                nc.vector.reciprocal(rs[:], sdf[:])
                neg = wpool.tile([128, 1], F32, tag="f_neg", name="f_neg")
                nc.vector.scalar_tensor_tensor(out=neg[:], in0=mu_[:], scalar=-1.0,
                                               in1=rs[:], op0=ALU.mult, op1=ALU.mult)
                outt = wpool.tile([128, 32], F32, tag="f_out", name="f_out")
                nc.scalar.activation(out=outt[:], in_=tot[:], func=AF.Identity,
                                     scale=rs[:, 0:1], bias=neg[:, 0:1])
                nc.sync.dma_start(out=out_mu[:, :], in_=outt[:])

    nc.compile()
    return nc


# ---------------------------------------------------------------- entry point
def kernel(**inputs) -> np.ndarray:
    per_core, meta = _host_prep(inputs)
    if "prog" not in _CACHE:
        _CACHE["prog"] = _build_nc(meta, per_core[0])
    nc = _CACHE["prog"]
    res = bass_utils.run_bass_kernel_spmd(nc, per_core, core_ids=list(range(NCORES)))
    return np.ascontiguousarray(res.results[0]["mu"].T)
